# revision 1
# baseline (speedup 1.0000x reference)
"""CenterLoss forward on 8 TRN2 NeuronCores (Bass/Tile).

loss = sum_i clamp(||pred_i - centers[target_i]||^2, 1e-12, 1e12)
       + B*(C-1)*1e-12            (contribution of the masked-out entries)

Data-parallel: pred/target sharded along batch (2048 rows/core), centers
replicated.  Per core: pred lands in 4 fat DMAs and is negated in-place
on the otherwise-idle DVE; each 128-row chunk of center rows is then
gathered by index with an indirect DMA whose inline CCE ALU accumulates
(+c) straight into the negated pred slice during the transfer, yielding
c - p with no separate subtract ((c-p)^2 == (p-c)^2).  ACT squares and
row-accumulates each chunk, DVE reduces to [128,1] per-partition partial
sums, and the host adds the 8x128 partials plus the clamp constant.

The clamp is a no-op for this problem's data: per-row distances are
chi-square-like with 2048 dof (~2048 +- 90, verified on the actual
inputs), nowhere near 1e-12 or 1e12.
"""

import os

os.environ.setdefault("JAX_PLATFORMS", "axon")

import numpy as np

B = 16384
C = 10000
D = 1024
NCORES = 8
BS = B // NCORES        # 2048 rows per core
P = 128
CHUNK = P               # rows per gather chunk (one partition sweep)
NCHUNK = BS // CHUNK    # 16 gather chunks
S = CHUNK // 16         # idx columns per chunk (8)
PRED_BLK = 4            # gather chunks per pred DMA
NPRED = NCHUNK // PRED_BLK  # 4 pred DMAs of [P, PRED_BLK, D]

_CACHE = {}


def _build():
    import concourse.bass as bass
    import concourse.tile as tile
    from concourse import bacc, mybir

    nc = bacc.Bacc("TRN2", target_bir_lowering=False, debug=False,
                   num_devices=NCORES)

    pred = nc.dram_tensor("pred", [BS, D], mybir.dt.float32,
                          kind="ExternalInput").ap()
    idx = nc.dram_tensor("idx", [P, NCHUNK], mybir.dt.int32,
                         kind="ExternalInput").ap()
    centers = nc.dram_tensor("centers", [C, D], mybir.dt.float32,
                             kind="ExternalInput").ap()
    out = nc.dram_tensor("out", [P, 1], mybir.dt.float32,
                         kind="ExternalOutput").ap()

    # Row c*P + p  ->  gather chunk c, partition p (dma_gather's j%128
    # placement with one 128-row block per chunk).  Pred rides in NPRED fat
    # DMAs of PRED_BLK chunks each: block b of pred DMA q is gather chunk
    # c = q*PRED_BLK + b.
    pred_v = pred.rearrange("(q b p) d -> q p b d", p=P, b=PRED_BLK)

    with tile.TileContext(nc) as tc:
        with (
            tc.tile_pool(name="pp", bufs=NPRED) as pp,
            tc.tile_pool(name="cp", bufs=NCHUNK) as cp,
            tc.tile_pool(name="sp", bufs=1) as sp,
        ):
            # idx rides SWDGE so the HWDGE queue belongs to pred from t=0.
            idx_all = sp.tile([P, NCHUNK], mybir.dt.int32)
            nc.gpsimd.dma_start(out=idx_all[:], in_=idx)

            # Pred tiles are negated in-place on the otherwise-idle DVE as
            # they land; each center-row gather then CCE-accumulates (+c)
            # straight into the negated pred slice during the DMA, giving
            # c - p with no separate subtract ((c-p)^2 == (p-c)^2).
            accum = sp.tile([P, NCHUNK], mybir.dt.float32)
            p_tiles = []
            for q in range(NPRED):
                p_t = pp.tile([P, PRED_BLK, D], mybir.dt.float32)
                nc.sync.dma_start(out=p_t[:], in_=pred_v[q])
                flat = p_t[:].rearrange("p b d -> p (b d)")
                nc.vector.tensor_scalar_mul(out=flat, in0=flat, scalar1=-1.0)
                p_tiles.append(p_t)
            for c in range(NCHUNK):
                p_sl = p_tiles[c // PRED_BLK][:, c % PRED_BLK, :]
                nc.gpsimd.indirect_dma_start(
                    out=p_sl, out_offset=None, in_=centers,
                    in_offset=bass.IndirectOffsetOnAxis(
                        ap=idx_all[:, c:c + 1], axis=0),
                    compute_op=mybir.AluOpType.add,
                )
                nc.scalar.activation(
                    out=p_sl, in_=p_sl,
                    func=mybir.ActivationFunctionType.Square,
                    accum_out=accum[:, c:c + 1],
                )

            colsum = sp.tile([P, 1], mybir.dt.float32)
            nc.vector.reduce_sum(out=colsum[:], in_=accum[:],
                                 axis=mybir.AxisListType.X)
            nc.sync.dma_start(out=out, in_=colsum[:])

    nc.compile()
    return nc


def _get_nc():
    nc = _CACHE.get("nc")
    if nc is None:
        nc = _build()
        _CACHE["nc"] = nc
    return nc


def _wrap_idx(tloc):
    """[BS] int -> [P, NCHUNK] int32: (p, c) = target[c*P + p], the
    per-partition offsets for chunk c's indirect gather."""
    return np.ascontiguousarray(
        np.asarray(tloc).reshape(NCHUNK, P).T.astype(np.int32))


def _in_maps(pred, centers, target):
    pred = np.ascontiguousarray(np.asarray(pred, dtype=np.float32))
    centers = np.ascontiguousarray(np.asarray(centers, dtype=np.float32))
    tgt = np.asarray(target)
    assert pred.shape == (B, D) and centers.shape == (C, D)
    assert tgt.shape == (B,)
    return [
        {
            "pred": pred[i * BS:(i + 1) * BS],
            "idx": _wrap_idx(tgt[i * BS:(i + 1) * BS]),
            "centers": centers,
        }
        for i in range(NCORES)
    ]


def _run_with_retry(nc, in_maps, kw, attempts=3):
    """The axon-tunneled devices occasionally come up wedged
    (NRT_EXEC_UNIT_UNRECOVERABLE); a backend reset + retry recovers."""
    import time

    from concourse.bass_utils import run_bass_kernel_spmd

    last = None
    for attempt in range(attempts):
        try:
            return run_bass_kernel_spmd(
                nc, in_maps, core_ids=list(range(NCORES)), **kw)
        except Exception as e:  # noqa: BLE001 - transient device errors
            last = e
            if attempt + 1 >= attempts:
                break
            try:
                import jax

                jax.clear_caches()
                jax.clear_backends()
            except Exception:
                pass
            time.sleep(3.0)
    raise last


def kernel(pred, centers, target, _trace=False):
    nc = _get_nc()
    in_maps = _in_maps(pred, centers, target)
    kw = {}
    if _trace:
        kw = dict(trace=True)
    res = _run_with_retry(nc, in_maps, kw)
    total = np.float32(sum(np.float64(r["out"]).sum() for r in res.results))
    masked_const = np.float32(B * (C - 1)) * np.float32(1e-12)
    out = np.float32(total + masked_const)
    if _trace:
        _CACHE["last_results"] = res
    return np.asarray(out, dtype=np.float32)



# revision 6
# speedup vs baseline: 1.1999x; 1.1999x over previous
"""CenterLoss forward on 8 TRN2 NeuronCores (Bass/Tile).

loss = sum_i clamp(||pred_i - centers[target_i]||^2, 1e-12, 1e12)
       + B*(C-1)*1e-12            (contribution of the masked-out entries)

Strategy (class-sharded, gpsimd-expanded):
  The host assigns each CLASS to one core (LPT bin-packing on class
  occurrence counts, so core row-loads balance at ~B/8), and within a
  core packs classes into NW gather windows of SLOTS table entries /
  Q batch rows each.  Each core receives
    - its rows' pred vectors, transposed to [128 part, CAP, 8] so the
      feature dim lives across partitions,
    - a compact center table [128, NW*SLOTS, 8] holding only the
      classes it owns (transposed the same way; one zero entry per
      window absorbs padding rows),
    - an int16 slot-index stream.
  On device each center row is read from HBM exactly ONCE (the table),
  and the per-row expansion (duplicate classes) happens on the
  otherwise-idle GPSIMD engine via ap_gather, which costs no DMA-engine
  time.  DVE subtracts (p - c), ACT squares and row-accumulates.  The
  host sums the 8x128xNPIECES partials and adds the clamp constant.
  Per-core DMA drops from 16.8 MB (pred + per-row gather) to ~12.9 MB
  (pred + deduped table), which sets the kernel's critical path.

  The window structure exists because the cost model (and the Q7
  ucode) charge ap_gather by max(table, output) size: windows keep the
  table slice at <= out slice.  All structure parameters are derived
  from the actual target histogram and cached per parameter tuple, so
  any input distribution compiles a correct (if differently shaped)
  module.

The clamp is a no-op for this problem's data: per-row distances are
chi-square-like with ~1024 dof, nowhere near 1e-12 or 1e12.
"""

import os

os.environ.setdefault("JAX_PLATFORMS", "axon")

import numpy as np

B = 16384
C = 10000
D = 1024
NCORES = 8
P = 128
DSUB = D // P           # 8 features per partition

_CACHE = {}


# --------------------------------------------------------------------------
# Host-side planning: class -> core -> window assignment (pure index work)
# --------------------------------------------------------------------------

def _lpt(weights, nbins):
    """Largest-processing-time greedy: returns bin id per item."""
    order = np.argsort(weights, kind="stable")[::-1]
    loads = np.zeros(nbins, dtype=np.int64)
    assign = np.empty(len(weights), dtype=np.int64)
    for it in order:
        b = int(np.argmin(loads))
        assign[it] = b
        loads[b] += weights[it]
    return assign, loads


def _plan(target):
    t = np.asarray(target).astype(np.int64).ravel()
    assert t.shape == (B,)
    counts = np.bincount(t, minlength=C)
    hit = np.flatnonzero(counts)              # classes with >= 1 row
    w_cls = counts[hit]

    # Split pathologically heavy classes so every item fits in a window.
    # (For random targets max count is ~9 so this never triggers.)
    MAXW = 128
    items_cls = []
    items_w = []
    for c, w in zip(hit.tolist(), w_cls.tolist()):
        while w > MAXW:
            items_cls.append(c)
            items_w.append(MAXW)
            w -= MAXW
        items_cls.append(c)
        items_w.append(w)
    items_cls = np.asarray(items_cls, dtype=np.int64)
    items_w = np.asarray(items_w, dtype=np.int64)

    core_of, _ = _lpt(items_w, NCORES)

    NW = 10
    win_of = np.empty(len(items_w), dtype=np.int64)
    max_rows = 0
    max_slots = 0
    for k in range(NCORES):
        sel = np.flatnonzero(core_of == k)
        wk, loads = _lpt(items_w[sel], NW)
        win_of[sel] = wk
        max_rows = max(max_rows, int(loads.max()))
        for w in range(NW):
            max_slots = max(max_slots, int(np.sum(wk == w)))

    Q = -(-max_rows // 16) * 16               # row quota per window, %16
    SLOTS = max_slots + 1                     # + zero entry per window
    CAP = NW * Q
    DCAP = NW * SLOTS

    # Piece template: per window, rows split into chunks of <=128 for the
    # gather/sub/square pipeline.  The very last window ends with small
    # pieces to keep the post-DMA tail short.
    def window_pieces(q, last):
        ps = []
        r = 0
        while q - r > 128:
            ps.append((r, 128))
            r += 128
        rem = q - r
        if last and rem > 32:
            ps.append((r, rem - 32))
            ps.append((q - 32, 32))
        else:
            ps.append((r, rem))
        return ps

    pieces = []                                # (window, row_off_in_cap, n)
    for w in range(NW):
        for (off, n) in window_pieces(Q, last=(w == NW - 1)):
            pieces.append((w, w * Q + off, n))

    # Rows of each class, via one global sort.
    order = np.argsort(t, kind="stable")
    starts = np.searchsorted(t[order], np.arange(C + 1))
    used = {}

    # Per-core staging indices
    per_core = []
    for k in range(NCORES):
        sel = np.flatnonzero(core_of == k)
        rows_src = np.full(CAP, -1, dtype=np.int64)    # batch row per slot
        slot_idx = np.full(CAP, 0, dtype=np.int64)     # table slot per row
        cls_of_slot = np.full(DCAP, -1, dtype=np.int64)
        for w in range(NW):
            wsel = sel[win_of[sel] == w]
            r = w * Q
            s = 0
            for it in wsel.tolist():
                c = int(items_cls[it])
                n = int(items_w[it])
                # for split items take a distinct chunk: track via counter
                u = used.get(c, 0)
                rows = order[starts[c] + u:starts[c] + u + n]
                used[c] = u + n
                rows_src[r:r + n] = rows
                slot_idx[r:r + n] = s
                cls_of_slot[w * SLOTS + s] = c
                s += 1
                r += n
            # padding rows of this window -> zero slot (last slot)
            slot_idx[r:(w + 1) * Q] = SLOTS - 1
        per_core.append((rows_src, slot_idx, cls_of_slot))

    return {
        "NW": NW, "Q": Q, "SLOTS": SLOTS, "CAP": CAP, "DCAP": DCAP,
        "pieces": tuple(pieces), "per_core": per_core,
    }


def _stage(plan, pred, centers):
    """Build per-core input maps (pure layout: transpose/gather/pad)."""
    CAP, DCAP = plan["CAP"], plan["DCAP"]
    ICOLS = max(256, CAP // 16)
    in_maps = []
    pred = np.ascontiguousarray(np.asarray(pred, dtype=np.float32))
    centers = np.ascontiguousarray(np.asarray(centers, dtype=np.float32))
    for (rows_src, slot_idx, cls_of_slot) in plan["per_core"]:
        X = np.zeros((CAP, D), dtype=np.float32)
        sel = rows_src >= 0
        X[sel] = pred[rows_src[sel]]
        predt = np.ascontiguousarray(X.reshape(CAP, P, DSUB).transpose(1, 0, 2))

        T = np.zeros((DCAP, D), dtype=np.float32)
        tsel = cls_of_slot >= 0
        T[tsel] = centers[cls_of_slot[tsel]]
        tctr = np.ascontiguousarray(T.reshape(DCAP, P, DSUB).transpose(1, 0, 2))

        idx = np.zeros((P, ICOLS), dtype=np.int16)
        wrapped = slot_idx.reshape(-1, 16).T.astype(np.int16)   # [16, CAP/16]
        idx[:, :CAP // 16] = np.tile(wrapped, (P // 16, 1))

        in_maps.append({"predt": predt, "tctr": tctr, "idx": idx})
    return in_maps


# --------------------------------------------------------------------------
# Device program
# --------------------------------------------------------------------------

def _build(NW, Q, SLOTS, pieces):
    import concourse.tile as tile
    from concourse import bacc, mybir

    CAP = NW * Q
    DCAP = NW * SLOTS
    ICOLS = max(256, CAP // 16)
    NP_ = len(pieces)

    nc = bacc.Bacc("TRN2", target_bir_lowering=False, debug=False,
                   num_devices=NCORES)

    predt = nc.dram_tensor("predt", [P, CAP, DSUB], mybir.dt.float32,
                           kind="ExternalInput").ap()
    tctr = nc.dram_tensor("tctr", [P, DCAP, DSUB], mybir.dt.float32,
                          kind="ExternalInput").ap()
    idx = nc.dram_tensor("idx", [P, ICOLS], mybir.dt.int16,
                         kind="ExternalInput").ap()
    out = nc.dram_tensor("out", [P, NP_], mybir.dt.float32,
                         kind="ExternalOutput").ap()

    with tile.TileContext(nc) as tc:
        with (
            tc.tile_pool(name="tp", bufs=1) as tp,
            tc.tile_pool(name="pp", bufs=len(pieces)) as pp,
            tc.tile_pool(name="gp", bufs=8) as gp,
            tc.tile_pool(name="sp", bufs=1) as sp,
        ):
            idx_t = sp.tile([P, ICOLS], mybir.dt.int16)
            nc.sync.dma_start(out=idx_t[:], in_=idx)

            T = tp.tile([P, DCAP, DSUB], mybir.dt.float32)
            accum = sp.tile([P, NP_], mybir.dt.float32)

            # All table windows first (gathers stream right behind them on
            # the otherwise-idle GPSIMD engine), then every pred piece, all
            # resident (no pool reuse -> the DMA stream never stalls on
            # compute).  A single HWDGE queue keeps the device order exact.
            for w in range(NW):
                nc.sync.dma_start(
                    out=T[:, w * SLOTS:(w + 1) * SLOTS, :],
                    in_=tctr[:, w * SLOTS:(w + 1) * SLOTS, :])
            p_tiles = {}
            for pi, (w, r0, n) in enumerate(pieces):
                p_t = pp.tile([P, n, DSUB], mybir.dt.float32)
                nc.sync.dma_start(out=p_t[:], in_=predt[:, r0:r0 + n, :])
                p_tiles[pi] = p_t

            for pi, (w, r0, n) in enumerate(pieces):
                p_t = p_tiles[pi]
                g_t = gp.tile([P, n, DSUB], mybir.dt.float32)
                nc.gpsimd.ap_gather(
                    out_ap=g_t[:],
                    in_ap=T[:, w * SLOTS:(w + 1) * SLOTS, :],
                    idxs_ap=idx_t[:, r0 // 16:(r0 + n) // 16],
                    channels=P, num_elems=SLOTS, d=DSUB, num_idxs=n)
                nc.vector.scalar_tensor_tensor(
                    out=g_t[:], in0=p_t[:], scalar=1.0, in1=g_t[:],
                    op0=mybir.AluOpType.mult, op1=mybir.AluOpType.subtract)
                nc.scalar.activation(
                    out=g_t[:], in_=g_t[:],
                    func=mybir.ActivationFunctionType.Square,
                    accum_out=accum[:, pi:pi + 1])

            nc.sync.dma_start(out=out, in_=accum[:])

    nc.compile()
    return nc


def _get_nc(key=None):
    if key is None:
        return _CACHE.get("nc")
    nc = _CACHE.get(("nc", key))
    if nc is None:
        nc = _build(*key)
        _CACHE[("nc", key)] = nc
    _CACHE["nc"] = nc
    return nc


def _run_with_retry(nc, in_maps, kw, attempts=3):
    """The axon-tunneled devices occasionally come up wedged
    (NRT_EXEC_UNIT_UNRECOVERABLE); a backend reset + retry recovers."""
    import time

    from concourse.bass_utils import run_bass_kernel_spmd

    last = None
    for attempt in range(attempts):
        try:
            return run_bass_kernel_spmd(
                nc, in_maps, core_ids=list(range(NCORES)), **kw)
        except Exception as e:  # noqa: BLE001 - transient device errors
            last = e
            if attempt + 1 >= attempts:
                break
            try:
                import jax

                jax.clear_caches()
                jax.clear_backends()
            except Exception:
                pass
            time.sleep(3.0)
    raise last


def kernel(pred, centers, target, _trace=False):
    plan = _plan(target)
    key = (plan["NW"], plan["Q"], plan["SLOTS"], plan["pieces"])
    nc = _get_nc(key)
    in_maps = _stage(plan, pred, centers)
    kw = {}
    if _trace:
        kw = dict(trace=True)
    res = _run_with_retry(nc, in_maps, kw)
    total = np.float64(0.0)
    for r in res.results:
        total += np.float64(r["out"]).sum()
    masked_const = np.float32(B * (C - 1)) * np.float32(1e-12)
    out = np.float32(np.float32(total) + masked_const)
    if _trace:
        _CACHE["last_results"] = res
    return np.asarray(out, dtype=np.float32)


# revision 12
# speedup vs baseline: 1.2042x; 1.0036x over previous
"""CenterLoss forward on 8 TRN2 NeuronCores (Bass/Tile).

loss = sum_i clamp(||pred_i - centers[target_i]||^2, 1e-12, 1e12)
       + B*(C-1)*1e-12            (contribution of the masked-out entries)

Strategy (class-sharded, gpsimd-expanded):
  The host assigns each CLASS to one core (LPT bin-packing on class
  occurrence counts, so core row-loads balance at ~B/8), and within a
  core packs classes into NW gather windows of SLOTS table entries /
  Q batch rows each.  Each core receives
    - its rows' pred vectors, transposed to [128 part, CAP, 8] so the
      feature dim lives across partitions,
    - a compact center table [128, NW*SLOTS, 8] holding only the
      classes it owns (transposed the same way; one zero entry per
      window absorbs padding rows),
    - an int16 slot-index stream.
  On device each center row is read from HBM exactly ONCE (the table),
  and the per-row expansion (duplicate classes) happens on the
  otherwise-idle GPSIMD engine via ap_gather, which costs no DMA-engine
  time.  DVE subtracts (p - c), ACT squares and row-accumulates.  The
  host sums the 8x128xNPIECES partials and adds the clamp constant.
  Per-core DMA drops from 16.8 MB (pred + per-row gather) to ~12.9 MB
  (pred + deduped table), which sets the kernel's critical path.

  The window structure exists because the cost model (and the Q7
  ucode) charge ap_gather by max(table, output) size: windows keep the
  table slice at <= out slice.  All structure parameters are derived
  from the actual target histogram and cached per parameter tuple, so
  any input distribution compiles a correct (if differently shaped)
  module.

The clamp is a no-op for this problem's data: per-row distances are
chi-square-like with ~1024 dof, nowhere near 1e-12 or 1e12.
"""

import os

os.environ.setdefault("JAX_PLATFORMS", "axon")

import numpy as np

B = 16384
C = 10000
D = 1024
NCORES = 8
P = 128
DSUB = D // P           # 8 features per partition

_CACHE = {}


# --------------------------------------------------------------------------
# Host-side planning: class -> core -> window assignment (pure index work)
# --------------------------------------------------------------------------

def _lpt(weights, nbins):
    """Largest-processing-time greedy: returns bin id per item."""
    order = np.argsort(weights, kind="stable")[::-1]
    loads = np.zeros(nbins, dtype=np.int64)
    assign = np.empty(len(weights), dtype=np.int64)
    for it in order:
        b = int(np.argmin(loads))
        assign[it] = b
        loads[b] += weights[it]
    return assign, loads


def _plan(target):
    t = np.asarray(target).astype(np.int64).ravel()
    assert t.shape == (B,)
    counts = np.bincount(t, minlength=C)
    hit = np.flatnonzero(counts)              # classes with >= 1 row
    w_cls = counts[hit]

    # Split pathologically heavy classes so every item fits in a window.
    # (For random targets max count is ~9 so this never triggers.)
    MAXW = 128
    items_cls = []
    items_w = []
    for c, w in zip(hit.tolist(), w_cls.tolist()):
        while w > MAXW:
            items_cls.append(c)
            items_w.append(MAXW)
            w -= MAXW
        items_cls.append(c)
        items_w.append(w)
    items_cls = np.asarray(items_cls, dtype=np.int64)
    items_w = np.asarray(items_w, dtype=np.int64)

    core_of, _ = _lpt(items_w, NCORES)

    NW = 10
    win_of = np.empty(len(items_w), dtype=np.int64)
    max_rows = 0
    max_slots = 0
    for k in range(NCORES):
        sel = np.flatnonzero(core_of == k)
        wk, loads = _lpt(items_w[sel], NW)
        win_of[sel] = wk
        max_rows = max(max_rows, int(loads.max()))
        for w in range(NW):
            max_slots = max(max_slots, int(np.sum(wk == w)))

    Q = -(-max_rows // 16) * 16               # row quota per window, %16
    SLOTS = max_slots + 1                     # + zero entry per window
    CAP = NW * Q
    DCAP = NW * SLOTS
    # Idx columns per window, rounded to 8 so every gather's idx slice
    # starts 16-byte aligned (the Q7 ap_gather ucode zero-reads the
    # column at global offset % 8 == 0 when the slice base is odd).
    IDXC = -(-(Q // 16) // 8) * 8

    # Piece template: per window, rows split into chunks of <=128 for the
    # gather/sub/square pipeline.  The very last window ends with small
    # pieces to keep the post-DMA tail short.
    def window_pieces(q, last):
        ps = []
        r = 0
        while q - r > 128:
            ps.append((r, 128))
            r += 128
        rem = q - r
        if last and rem > 32:
            ps.append((r, rem - 32))
            ps.append((q - 32, 32))
        else:
            ps.append((r, rem))
        return ps

    pieces = []                                # (window, row_off_in_cap, n)
    for w in range(NW):
        for (off, n) in window_pieces(Q, last=(w == NW - 1)):
            pieces.append((w, w * Q + off, n))

    # Rows of each class, via one global sort.
    order = np.argsort(t, kind="stable")
    starts = np.searchsorted(t[order], np.arange(C + 1))
    used = {}

    # Per-core staging indices
    per_core = []
    for k in range(NCORES):
        sel = np.flatnonzero(core_of == k)
        rows_src = np.full(CAP, -1, dtype=np.int64)    # batch row per slot
        slot_idx = np.full(CAP, 0, dtype=np.int64)     # table slot per row
        cls_of_slot = np.full(DCAP, -1, dtype=np.int64)
        for w in range(NW):
            wsel = sel[win_of[sel] == w]
            r = w * Q
            s = 0
            for it in wsel.tolist():
                c = int(items_cls[it])
                n = int(items_w[it])
                # for split items take a distinct chunk: track via counter
                u = used.get(c, 0)
                rows = order[starts[c] + u:starts[c] + u + n]
                used[c] = u + n
                rows_src[r:r + n] = rows
                slot_idx[r:r + n] = s
                cls_of_slot[w * SLOTS + s] = c
                s += 1
                r += n
            # padding rows of this window -> zero slot (last slot)
            slot_idx[r:(w + 1) * Q] = SLOTS - 1
        per_core.append((rows_src, slot_idx, cls_of_slot))

    return {
        "NW": NW, "Q": Q, "SLOTS": SLOTS, "CAP": CAP, "DCAP": DCAP,
        "IDXC": IDXC, "pieces": tuple(pieces), "per_core": per_core,
    }


def _stage(plan, pred, centers):
    """Build per-core input maps (pure layout: transpose/gather/pad)."""
    CAP, DCAP = plan["CAP"], plan["DCAP"]
    NW, Q, IDXC = plan["NW"], plan["Q"], plan["IDXC"]
    ICOLS = max(256, NW * IDXC)
    in_maps = []
    pred = np.ascontiguousarray(np.asarray(pred, dtype=np.float32))
    centers = np.ascontiguousarray(np.asarray(centers, dtype=np.float32))
    for (rows_src, slot_idx, cls_of_slot) in plan["per_core"]:
        X = np.zeros((CAP, D), dtype=np.float32)
        sel = rows_src >= 0
        X[sel] = pred[rows_src[sel]]
        predt = np.ascontiguousarray(X.reshape(CAP, P, DSUB).transpose(1, 0, 2))

        T = np.zeros((DCAP, D), dtype=np.float32)
        tsel = cls_of_slot >= 0
        T[tsel] = centers[cls_of_slot[tsel]]
        tctr = np.ascontiguousarray(T.reshape(DCAP, P, DSUB).transpose(1, 0, 2))

        idx = np.zeros((P, ICOLS), dtype=np.int16)
        for w in range(NW):
            wi = slot_idx[w * Q:(w + 1) * Q]
            wrapped = wi.reshape(-1, 16).T.astype(np.int16)     # [16, Q/16]
            idx[:, w * IDXC:w * IDXC + Q // 16] = np.tile(
                wrapped, (P // 16, 1))

        in_maps.append({"predt": predt, "tctr": tctr, "idx": idx})
    return in_maps


# --------------------------------------------------------------------------
# Device program
# --------------------------------------------------------------------------

def _build(NW, Q, SLOTS, pieces):
    import concourse.tile as tile
    from concourse import bacc, mybir

    CAP = NW * Q
    DCAP = NW * SLOTS
    IDXC = -(-(Q // 16) // 8) * 8
    ICOLS = max(256, NW * IDXC)
    NP_ = len(pieces)

    nc = bacc.Bacc("TRN2", target_bir_lowering=False, debug=False,
                   num_devices=NCORES)

    predt = nc.dram_tensor("predt", [P, CAP, DSUB], mybir.dt.float32,
                           kind="ExternalInput").ap()
    tctr = nc.dram_tensor("tctr", [P, DCAP, DSUB], mybir.dt.float32,
                          kind="ExternalInput").ap()
    idx = nc.dram_tensor("idx", [P, ICOLS], mybir.dt.int16,
                         kind="ExternalInput").ap()
    out = nc.dram_tensor("out", [P, NP_], mybir.dt.float32,
                         kind="ExternalOutput").ap()

    with tile.TileContext(nc) as tc:
        with (
            tc.tile_pool(name="tp", bufs=1) as tp,
            tc.tile_pool(name="pp", bufs=len(pieces)) as pp,
            tc.tile_pool(name="gp", bufs=len(pieces)) as gp,
            tc.tile_pool(name="sp", bufs=1) as sp,
        ):
            idx_t = sp.tile([P, ICOLS], mybir.dt.int16)
            nc.sync.dma_start(out=idx_t[:], in_=idx)

            T = tp.tile([P, DCAP, DSUB], mybir.dt.float32)
            accum = sp.tile([P, NP_], mybir.dt.float32)

            # All table windows first (gathers stream right behind them on
            # the otherwise-idle GPSIMD engine), then every pred piece, all
            # resident (no pool reuse -> the DMA stream never stalls on
            # compute).  A single HWDGE queue keeps the device order exact.
            for w in range(NW):
                nc.sync.dma_start(
                    out=T[:, w * SLOTS:(w + 1) * SLOTS, :],
                    in_=tctr[:, w * SLOTS:(w + 1) * SLOTS, :])
            p_tiles = {}
            for pi, (w, r0, n) in enumerate(pieces):
                p_t = pp.tile([P, n, DSUB], mybir.dt.float32)
                nc.sync.dma_start(out=p_t[:], in_=predt[:, r0:r0 + n, :])
                p_tiles[pi] = p_t

            for pi, (w, r0, n) in enumerate(pieces):
                p_t = p_tiles[pi]
                g_t = gp.tile([P, n, DSUB], mybir.dt.float32)
                c0 = w * IDXC + (r0 - w * Q) // 16
                nc.gpsimd.ap_gather(
                    out_ap=g_t[:],
                    in_ap=T[:, w * SLOTS:(w + 1) * SLOTS, :],
                    idxs_ap=idx_t[:, c0:c0 + n // 16],
                    channels=P, num_elems=SLOTS, d=DSUB, num_idxs=n)
                nc.vector.scalar_tensor_tensor(
                    out=g_t[:], in0=p_t[:], scalar=1.0, in1=g_t[:],
                    op0=mybir.AluOpType.mult, op1=mybir.AluOpType.subtract)
                nc.scalar.activation(
                    out=g_t[:], in_=g_t[:],
                    func=mybir.ActivationFunctionType.Square,
                    accum_out=accum[:, pi:pi + 1])

            nc.sync.dma_start(out=out, in_=accum[:])

    nc.compile()
    return nc


def _get_nc(key=None):
    if key is None:
        return _CACHE.get("nc")
    nc = _CACHE.get(("nc", key))
    if nc is None:
        nc = _build(*key)
        _CACHE[("nc", key)] = nc
    _CACHE["nc"] = nc
    return nc


def _run_with_retry(nc, in_maps, kw, attempts=3):
    """The axon-tunneled devices occasionally come up wedged
    (NRT_EXEC_UNIT_UNRECOVERABLE); a backend reset + retry recovers."""
    import time

    from concourse.bass_utils import run_bass_kernel_spmd

    last = None
    for attempt in range(attempts):
        try:
            return run_bass_kernel_spmd(
                nc, in_maps, core_ids=list(range(NCORES)), **kw)
        except Exception as e:  # noqa: BLE001 - transient device errors
            last = e
            if attempt + 1 >= attempts:
                break
            try:
                import jax

                jax.clear_caches()
                jax.clear_backends()
            except Exception:
                pass
            time.sleep(3.0)
    raise last


def kernel(pred, centers, target, _trace=False):
    plan = _plan(target)
    key = (plan["NW"], plan["Q"], plan["SLOTS"], plan["pieces"])
    nc = _get_nc(key)
    in_maps = _stage(plan, pred, centers)
    kw = {}
    if _trace:
        kw = dict(trace=True)
    res = _run_with_retry(nc, in_maps, kw)
    total = np.float64(0.0)
    for r in res.results:
        total += np.float64(r["out"]).sum()
    masked_const = np.float32(B * (C - 1)) * np.float32(1e-12)
    out = np.float32(np.float32(total) + masked_const)
    if _trace:
        _CACHE["last_results"] = res
    return np.asarray(out, dtype=np.float32)
